# revision 1
# baseline (speedup 1.0000x reference)
"""Trainium2 Bass kernel for nn_OneToOneLinear.

Computes sigmoid(SCALE * (input * weight + bias)) where input is
[N, F] f32, weight/bias are [F] f32 (per-feature), SCALE = 4.0.

Strategy: trivially data-parallel. Shard input rows across 8
NeuronCores ([4096, 2048] per core), replicate weight/bias. Per core:
stream [128, TILE_COLS] f32 tiles through SBUF — HWDGE load on the SP
ring, DVE mul (x*w) + DVE add (+b) in place, ACT sigmoid with
scale=4.0, HWDGE store issued from the ACT ring so stores never stall
the load sequencer. The problem is memory-bound: 64 MiB of HBM traffic
per core (~358 GB/s/NC HBM limit -> ~187 us roofline).
"""

import numpy as np

N = 32768
F = 2048
N_CORES = 8
ROWS = N // N_CORES  # rows per core
P = 128
SCALE = 4.0

# How many 128-row groups to pack into one SBUF tile / DMA transfer.
# TILE_COLS = KPACK * F bytes per partition; KPACK=2 -> 2 MiB DMAs.
KPACK = 2
BUFS = 6

_cache = {}


def _build_program():
    import concourse.bacc as bacc
    import concourse.bass as bass
    import concourse.mybir as mybir
    import concourse.tile as tile

    nc = bacc.Bacc(
        "TRN2",
        target_bir_lowering=False,
        debug=False,
        num_devices=N_CORES,
    )
    inp = nc.dram_tensor("input", [ROWS, F], mybir.dt.float32, kind="ExternalInput").ap()
    w = nc.dram_tensor("weight", [F], mybir.dt.float32, kind="ExternalInput").ap()
    b = nc.dram_tensor("bias", [F], mybir.dt.float32, kind="ExternalInput").ap()
    out = nc.dram_tensor("output", [ROWS, F], mybir.dt.float32, kind="ExternalOutput").ap()

    n_tiles = ROWS // (P * KPACK)
    cols = KPACK * F

    with tile.TileContext(nc) as tc:
        with (
            tc.tile_pool(name="consts", bufs=1) as consts,
            tc.tile_pool(name="io", bufs=BUFS) as pool,
        ):
            # Broadcast weight/bias to all 128 partitions once (step-0
            # leading dim on the DRAM side; SWDGE handles the replication).
            w_sb = consts.tile([P, F], mybir.dt.float32)
            b_sb = consts.tile([P, F], mybir.dt.float32)
            w_bc = bass.AP(tensor=w.tensor, offset=w.offset, ap=[[0, P], *w.ap])
            b_bc = bass.AP(tensor=b.tensor, offset=b.offset, ap=[[0, P], *b.ap])
            nc.gpsimd.dma_start(out=w_sb[:], in_=w_bc)
            nc.gpsimd.dma_start(out=b_sb[:], in_=b_bc)

            # [ROWS, F] -> [n_tiles, P, KPACK, F]: tile i, partition p,
            # free (k, f) <- row i*P*KPACK + k*P + p, feature f.
            inp_t = inp.rearrange("(t k p) f -> t p k f", p=P, k=KPACK)
            out_t = out.rearrange("(t k p) f -> t p k f", p=P, k=KPACK)

            for i in range(n_tiles):
                x = pool.tile([P, cols], mybir.dt.float32)
                x3 = x.rearrange("p (k f) -> p k f", k=KPACK)
                nc.sync.dma_start(out=x3, in_=inp_t[i])
                for k in range(KPACK):
                    xk = x[:, k * F : (k + 1) * F]
                    nc.vector.tensor_mul(out=xk, in0=xk, in1=w_sb[:])
                    nc.vector.tensor_add(out=xk, in0=xk, in1=b_sb[:])
                # sigmoid(4 * t), issued on ACT; store from the ACT ring.
                nc.scalar.activation(
                    x[:], x[:], mybir.ActivationFunctionType.Sigmoid, scale=SCALE
                )
                nc.scalar.dma_start(out=out_t[i], in_=x3)

    nc.compile()
    return nc


def kernel(input, weight, bias):
    from concourse.bass_utils import run_bass_kernel_spmd

    if "nc" not in _cache:
        _cache["nc"] = _build_program()
    nc = _cache["nc"]

    input = np.ascontiguousarray(np.asarray(input), dtype=np.float32)
    weight = np.ascontiguousarray(np.asarray(weight), dtype=np.float32)
    bias = np.ascontiguousarray(np.asarray(bias), dtype=np.float32)

    shards = np.split(input, N_CORES, axis=0)
    in_maps = [
        {"input": s, "weight": weight, "bias": bias} for s in shards
    ]
    res = run_bass_kernel_spmd(nc, in_maps, list(range(N_CORES))).results
    return np.concatenate([r["output"] for r in res], axis=0)
